# revision 31
# baseline (speedup 1.0000x reference)
"""AttentionPooling kernel for 8 TRN2 NeuronCores.

Strategy (feature-major, scan-based segment sum):
  - Host shards nodes across 8 cores at graph boundaries (graph_index is
    sorted), pads each graph's rows to a multiple of PAD, packs whole graphs
    into fixed-size chunks of C rows (chunk tails zero-padded), and
    pre-transposes inputs to feature-major bf16 layout. x1 chunk PAIRS are
    stacked on the partition axis ([0:64] even chunk, [64:128] odd chunk) so
    x1 DMAs use all 128 partitions; the matmuls address PE row groups.
  - Device (SPMD, identical program on 8 cores):
      att.T  = sigmoid(W1 @ x1.T + W2 @ x2.T + b1)      (PE + ACT, bf16 in)
      m2.T   = W3 @ x2.T + b3                           (PE + ACT)
      g.T    = att.T * m2.T                             (DVE, bf16 2x mode)
      r4     = 4-column pair-reduce of g.T              (DVE, fp32 out)
      dec    = prefix-scan of r4 along rows, per chunk  (DVE scan, fp32)
    Weights stay stationary across 4 consecutive matmuls (weight-outer
    ordering) so LDWEIGHTS amortizes. Because every graph starts/ends on a
    multiple of PAD rows, every graph boundary's cumulative sum is present
    in the decimated output.
  - Host extracts per-graph sums as differences of decimated scan values,
    corrects for in-graph padding rows (nonzero only if biases nonzero),
    zeroes empty graphs, and concatenates the per-core graph ranges.
"""

import numpy as np

NUM_GRAPHS = 50000
N_NODES = 1_000_000
MOL_C = 64
HID_C = 128
N_CORES = 8
GPC = NUM_GRAPHS // N_CORES          # graphs per core
PAD = 4                              # pad each graph's rows to multiple of PAD
C = 4096                             # rows per device chunk
DEC = C // PAD                       # decimated cols per chunk
NCHUNK_CAP = 40                      # sanity cap on chunks per core
NBLK = C // 1024                     # psum blocks per chunk

LAST_RESULTS = None                  # stash for profiling from test harness


def _build_bass(nchunk: int, need_b3: bool):
    import concourse.bacc as bacc
    import concourse.tile as tile
    from concourse import mybir

    f32 = mybir.dt.float32
    bf16 = mybir.dt.bfloat16
    nc = bacc.Bacc()

    rt = nchunk * C
    npair = (nchunk + 1) // 2
    x1t = nc.dram_tensor("x1t", [2 * MOL_C, npair * C], bf16,
                         kind="ExternalInput")
    x2t = nc.dram_tensor("x2t", [HID_C, rt], bf16, kind="ExternalInput")
    w1t = nc.dram_tensor("w1t", [2 * MOL_C, HID_C], bf16, kind="ExternalInput")
    w2t = nc.dram_tensor("w2t", [HID_C, HID_C], bf16, kind="ExternalInput")
    w3t = nc.dram_tensor("w3t", [HID_C, HID_C], bf16, kind="ExternalInput")
    b1 = nc.dram_tensor("b1", [HID_C, 1], f32, kind="ExternalInput")
    b3 = nc.dram_tensor("b3", [HID_C, 1], f32, kind="ExternalInput")
    dec = nc.dram_tensor("dec", [HID_C, nchunk * DEC], f32,
                         kind="ExternalOutput")

    Act = mybir.ActivationFunctionType
    Alu = mybir.AluOpType

    with tile.TileContext(nc) as tc:
        with (
            tc.tile_pool(name="const", bufs=1) as cp,
            tc.tile_pool(name="xin", bufs=3) as xp,
            tc.tile_pool(name="mid", bufs=2) as mp,
            tc.tile_pool(name="scan", bufs=2) as sp,
            tc.tile_pool(name="psum", bufs=2, space="PSUM") as pp,
        ):
            w1 = cp.tile([2 * MOL_C, HID_C], bf16)
            nc.sync.dma_start(out=w1[:], in_=w1t[:, :])
            w2 = cp.tile([HID_C, HID_C], bf16)
            nc.sync.dma_start(out=w2[:], in_=w2t[:, :])
            w3 = cp.tile([HID_C, HID_C], bf16)
            nc.sync.dma_start(out=w3[:], in_=w3t[:, :])
            b1s = cp.tile([HID_C, 1], f32)
            nc.sync.dma_start(out=b1s[:], in_=b1[:, :])
            b3s = cp.tile([HID_C, 1], f32)
            nc.sync.dma_start(out=b3s[:], in_=b3[:, :])

            # Prime engines on the freshly-DMA'd constants so no later
            # fused-LDW matmul needs two sync waits (walrus allows one).
            prime_ps = pp.tile([HID_C, 8], f32, tag="pa")
            prime_sb = cp.tile([HID_C, 8], f32)
            nc.tensor.matmul(prime_ps[:, 0:1], w1[:], w1[:, 0:1],
                             start=True, stop=True)
            nc.tensor.matmul(prime_ps[:, 1:2], w2[:], w2[:, 0:1],
                             start=True, stop=True)
            nc.tensor.matmul(prime_ps[:, 2:3], w3[:], w3[:, 0:1],
                             start=True, stop=True)
            nc.scalar.activation(prime_sb[:, 0:1], b1s[:, 0:1], Act.Copy)
            nc.scalar.activation(prime_sb[:, 1:2], b3s[:, 0:1], Act.Copy)

            x1d = None
            for ch in range(nchunk):
                par = ch % 2
                # chunk 0: split input DMAs so the first matmuls (and hence
                # the whole ACT->DVE pipeline) start as early as possible
                nsplit = 4 if ch == 0 else 1
                if par == 0:
                    x1d = xp.tile([2 * MOL_C, C], bf16, tag="x1",
                                  name=f"x1_{ch}")
                    pr = ch // 2
                    for sp0 in range(nsplit):
                        ssl = slice(sp0 * C // nsplit, (sp0 + 1) * C // nsplit)
                        dsl = slice(pr * C + sp0 * C // nsplit,
                                    pr * C + (sp0 + 1) * C // nsplit)
                        nc.sync.dma_start(out=x1d[:, ssl], in_=x1t[:, dsl])
                x1lo = slice(par * MOL_C, (par + 1) * MOL_C)
                x2 = xp.tile([HID_C, C], bf16, tag="x2", name=f"x2_{ch}")
                for sp0 in range(nsplit):
                    ssl = slice(sp0 * C // nsplit, (sp0 + 1) * C // nsplit)
                    dsl = slice(ch * C + sp0 * C // nsplit,
                                ch * C + (sp0 + 1) * C // nsplit)
                    nc.sync.dma_start(out=x2[:, ssl], in_=x2t[:, dsl])

                atts = mp.tile([HID_C, C], bf16, tag="atts",
                               name=f"atts_{ch}")
                m2s = mp.tile([HID_C, C], bf16, tag="m2s", name=f"m2s_{ch}")
                g = sp.tile([HID_C, C], bf16, tag="g", name=f"g_{ch}")
                # weight-outer ordering: each stationary weight serves 4
                # consecutive N=512 matmuls so LDWEIGHTS amortizes.
                for half in range(NBLK // 2):
                    blks = (2 * half, 2 * half + 1)
                    pas = [pp.tile([HID_C, 1024], f32, tag="pa",
                                   name=f"pa_{ch}_{half}_{i}")
                           for i in range(2)]
                    pms = [pp.tile([HID_C, 1024], f32, tag="pm",
                                   name=f"pm_{ch}_{half}_{i}")
                           for i in range(2)]
                    for wt, wsl, xt, xsl, outs, st, sp_ in (
                        (w1, x1lo, x1d, x1lo, pas, True, False),
                        (w2, slice(None), x2, slice(None), pas, False, True),
                        (w3, slice(None), x2, slice(None), pms, True, True),
                    ):
                        for i, blk in enumerate(blks):
                            for j in range(2):
                                sl = slice(blk * 1024 + j * 512,
                                           blk * 1024 + (j + 1) * 512)
                                ps = slice(j * 512, (j + 1) * 512)
                                nc.tensor.matmul(outs[i][:, ps], wt[wsl, :],
                                                 xt[xsl, sl],
                                                 start=st, stop=sp_)
                    for i, blk in enumerate(blks):
                        bsl = slice(blk * 1024, (blk + 1) * 1024)
                        nc.scalar.activation(atts[:, bsl], pas[i][:],
                                             Act.Sigmoid, bias=b1s[:, :1],
                                             scale=1.0)
                        if need_b3 or blk < 3:
                            # staged path: ACT casts pm to bf16 SBUF (+b3),
                            # DVE multiply runs in the 2x packed mode
                            nc.scalar.activation(m2s[:, bsl], pms[i][:],
                                                 Act.Identity,
                                                 bias=b3s[:, :1], scale=1.0)
                            if need_b3 and blk == 3 and ch > 0:
                                nc.vector.tensor_tensor(
                                    out=g[:, bsl], in0=atts[:, bsl],
                                    in1=m2s[:, bsl], op=Alu.mult)
                        else:
                            # balance path (b3==0): skip the ACT cast, DVE
                            # reads the matmul PSUM directly at 1x
                            nc.vector.tensor_tensor(out=g[:, bsl],
                                                    in0=atts[:, bsl],
                                                    in1=pms[i][:],
                                                    op=Alu.mult)
                        if ch == 0 and (need_b3 or blk < 3):
                            # ramp: per-block multiplies so DVE starts early
                            nc.vector.tensor_tensor(out=g[:, bsl],
                                                    in0=atts[:, bsl],
                                                    in1=m2s[:, bsl],
                                                    op=Alu.mult)
                        elif ch > 0 and blk == 2:
                            # one merged 2x multiply for blocks 0-2
                            msl = slice(0, 3 * 1024)
                            nc.vector.tensor_tensor(out=g[:, msl],
                                                    in0=atts[:, msl],
                                                    in1=m2s[:, msl],
                                                    op=Alu.mult)

                # The host places the 4 rows of decimation group j at
                # columns j, j+DEC, j+2*DEC, j+3*DEC, so the 4-to-1
                # pair-reduce is two contiguous-half adds. The first level
                # runs on the DMA engines (SBUF->SBUF copy + CCE-add
                # accumulate), freeing the vector engine.
                r2 = sp.tile([HID_C, C // 2], bf16, tag="r2", name=f"r2_{ch}")
                nc.gpsimd.dma_start(out=r2[:], in_=g[:, :C // 2])
                nc.gpsimd.dma_start(out=r2[:], in_=g[:, C // 2:],
                                    accum_op=Alu.add)
                r4 = sp.tile([HID_C, DEC], f32, tag="r4", name=f"r4_{ch}")
                nc.vector.tensor_tensor(out=r4[:], in0=r2[:, :DEC],
                                        in1=r2[:, DEC:], op=Alu.add)
                dtile = sp.tile([HID_C, DEC], f32, tag="dt", name=f"dt_{ch}")
                nc.vector.tensor_tensor_scan(
                    out=dtile[:], data0=r4[:], data1=r4[:], initial=0.0,
                    op0=Alu.add, op1=Alu.bypass,
                )
                nc.sync.dma_start(out=dec[:, ch * DEC:(ch + 1) * DEC],
                                  in_=dtile[:])
    nc.compile()
    return nc


def kernel(input_rep, final_rep, graph_index, lin_w, lin_b, last_w, last_b):
    global LAST_RESULTS
    import ml_dtypes
    from concourse.bass_utils import run_bass_kernel_spmd

    bf16 = ml_dtypes.bfloat16
    x1 = np.ascontiguousarray(np.asarray(input_rep, dtype=np.float32))
    x2 = np.ascontiguousarray(np.asarray(final_rep, dtype=np.float32))
    gi = np.asarray(graph_index).astype(np.int64)
    lw = np.asarray(lin_w, dtype=np.float32)
    lb = np.asarray(lin_b, dtype=np.float32)
    tw = np.asarray(last_w, dtype=np.float32)
    tb = np.asarray(last_b, dtype=np.float32)

    counts = np.bincount(gi, minlength=NUM_GRAPHS).astype(np.int64)
    pc = ((counts + PAD - 1) // PAD) * PAD          # padded per-graph rows
    row_begin = np.concatenate([[0], np.cumsum(counts)])  # src row offsets

    # per-core greedy chunk packing of whole (padded) graphs
    packing = []
    nchunk = 0
    for k in range(N_CORES):
        glo, ghi = k * GPC, (k + 1) * GPC
        pk = pc[glo:ghi]
        chunk_id = np.empty(GPC, dtype=np.int64)
        local_start = np.empty(GPC, dtype=np.int64)
        cum = 0
        ch = 0
        for i in range(GPC):
            p = pk[i]
            if cum + p > C:
                ch += 1
                cum = 0
            chunk_id[i] = ch
            local_start[i] = cum
            cum += p
        packing.append((chunk_id, local_start))
        nchunk = max(nchunk, ch + 1)
    assert nchunk <= NCHUNK_CAP, f"needs {nchunk} chunks > {NCHUNK_CAP}"
    rt = nchunk * C
    npair = (nchunk + 1) // 2

    need_b3 = bool(np.any(tb != 0.0))
    nc = _build_bass(nchunk, need_b3)

    w1t = np.zeros((2 * MOL_C, HID_C), dtype=bf16)
    w1t[:MOL_C] = lw[:, :MOL_C].T.astype(bf16)
    w1t[MOL_C:] = w1t[:MOL_C]
    w2t = np.ascontiguousarray(lw[:, MOL_C:].T).astype(bf16)
    w3t = np.ascontiguousarray(tw.T).astype(bf16)
    b1v = np.ascontiguousarray(lb.reshape(HID_C, 1))
    b3v = np.ascontiguousarray(tb.reshape(HID_C, 1))

    in_maps = []
    ext = []
    for k in range(N_CORES):
        glo, ghi = k * GPC, (k + 1) * GPC
        ck = counts[glo:ghi]
        pk = pc[glo:ghi]
        chunk_id, local_start = packing[k]

        # destination rows for real node rows
        nk = int(ck.sum())
        dst_base = chunk_id * C + local_start
        src0 = row_begin[glo]
        within = np.arange(src0, src0 + nk) - np.repeat(row_begin[glo:ghi], ck)
        dst = np.repeat(dst_base, ck) + within
        # column permutation: row L of a chunk lands at column
        # (L//PAD) + (L%PAD)*DEC so the pair-reduce reads contiguous halves
        lc = dst % C
        dst = (dst - lc) + (lc // PAD) + (lc % PAD) * DEC

        # x1: chunk pairs stacked along the partition axis
        x1t = np.zeros((2 * MOL_C, npair * C), dtype=bf16)
        dch = dst // C
        dcol = (dch // 2) * C + (dst % C)
        drow = (dch % 2) * MOL_C
        x1v = x1[src0:src0 + nk].T.astype(bf16)       # [64, nk]
        even = drow == 0
        x1t[:MOL_C, dcol[even]] = x1v[:, even]
        x1t[MOL_C:, dcol[~even]] = x1v[:, ~even]

        x2t = np.zeros((HID_C, rt), dtype=bf16)
        x2t[:, dst] = x2[src0:src0 + nk].T.astype(bf16)

        in_maps.append({
            "x1t": x1t, "x2t": x2t, "w1t": w1t, "w2t": w2t, "w3t": w3t,
            "b1": b1v, "b3": b3v,
        })
        ext.append((ck, pk, chunk_id, local_start))

    res = run_bass_kernel_spmd(nc, in_maps, core_ids=list(range(N_CORES)))
    LAST_RESULTS = res

    # pad-row gated value (zero when biases are zero)
    pad_g = (1.0 / (1.0 + np.exp(-lb))) * tb          # [HID_C]

    out = np.empty((NUM_GRAPHS, HID_C), dtype=np.float32)
    for k in range(N_CORES):
        deck = np.asarray(res.results[k]["dec"])      # [HID_C, nchunk*DEC]
        ck, pk, chunk_id, local_start = ext[k]
        end_col = chunk_id * DEC + (local_start + pk) // PAD - 1
        start_col = chunk_id * DEC + local_start // PAD - 1
        e = deck[:, end_col]                          # [HID_C, GPC]
        s = deck[:, start_col]
        s[:, local_start == 0] = 0.0
        o = (e - s).T                                 # [GPC, HID_C]
        o -= (pk - ck)[:, None].astype(np.float32) * pad_g[None, :]
        o[ck == 0] = 0.0
        out[k * GPC:(k + 1) * GPC] = o
    return out


# revision 32
# speedup vs baseline: 1.5549x; 1.5549x over previous
"""AttentionPooling kernel for 8 TRN2 NeuronCores.

Strategy (feature-major, scan-based segment sum):
  - Host shards nodes across 8 cores at graph boundaries (graph_index is
    sorted), pads each graph's rows to a multiple of PAD, packs whole graphs
    into fixed-size chunks of C rows (chunk tails zero-padded), and
    pre-transposes inputs to feature-major bf16 layout. x1 chunk PAIRS are
    stacked on the partition axis ([0:64] even chunk, [64:128] odd chunk) so
    x1 DMAs use all 128 partitions; the matmuls address PE row groups.
  - Device (SPMD, identical program on 8 cores):
      att.T  = sigmoid(W1 @ x1.T + W2 @ x2.T + b1)      (PE + ACT, bf16 in)
      m2.T   = W3 @ x2.T + b3                           (PE + ACT)
      g.T    = att.T * m2.T                             (DVE, bf16 2x mode)
      r4     = 4-column pair-reduce of g.T              (DVE, fp32 out)
      dec    = prefix-scan of r4 along rows, per chunk  (DVE scan, fp32)
    Weights stay stationary across 4 consecutive matmuls (weight-outer
    ordering) so LDWEIGHTS amortizes. Because every graph starts/ends on a
    multiple of PAD rows, every graph boundary's cumulative sum is present
    in the decimated output.
  - Host extracts per-graph sums as differences of decimated scan values,
    corrects for in-graph padding rows (nonzero only if biases nonzero),
    zeroes empty graphs, and concatenates the per-core graph ranges.
"""

import numpy as np

NUM_GRAPHS = 50000
N_NODES = 1_000_000
MOL_C = 64
HID_C = 128
N_CORES = 8
GPC = NUM_GRAPHS // N_CORES          # graphs per core
PAD = 4                              # pad each graph's rows to multiple of PAD
C = 4096                             # rows per device chunk
DEC = C // PAD                       # decimated cols per chunk
NCHUNK_CAP = 40                      # sanity cap on chunks per core
NBLK = C // 1024                     # psum blocks per chunk

LAST_RESULTS = None                  # stash for profiling from test harness


def _build_bass(nchunk: int, need_b3: bool):
    import concourse.bacc as bacc
    import concourse.tile as tile
    from concourse import mybir

    f32 = mybir.dt.float32
    bf16 = mybir.dt.bfloat16
    nc = bacc.Bacc()

    rt = nchunk * C
    npair = (nchunk + 1) // 2
    x1t = nc.dram_tensor("x1t", [2 * MOL_C, npair * C], bf16,
                         kind="ExternalInput")
    x2t = nc.dram_tensor("x2t", [HID_C, rt], bf16, kind="ExternalInput")
    w1t = nc.dram_tensor("w1t", [2 * MOL_C, HID_C], bf16, kind="ExternalInput")
    w2t = nc.dram_tensor("w2t", [HID_C, HID_C], bf16, kind="ExternalInput")
    w3t = nc.dram_tensor("w3t", [HID_C, HID_C], bf16, kind="ExternalInput")
    b1 = nc.dram_tensor("b1", [HID_C, 1], f32, kind="ExternalInput")
    b3 = nc.dram_tensor("b3", [HID_C, 1], f32, kind="ExternalInput")
    dec = nc.dram_tensor("dec", [HID_C, nchunk * DEC], f32,
                         kind="ExternalOutput")

    Act = mybir.ActivationFunctionType
    Alu = mybir.AluOpType

    with tile.TileContext(nc) as tc:
        with (
            tc.tile_pool(name="const", bufs=1) as cp,
            tc.tile_pool(name="xin", bufs=3) as xp,
            tc.tile_pool(name="mid", bufs=2) as mp,
            tc.tile_pool(name="scan", bufs=2) as sp,
            tc.tile_pool(name="psum", bufs=2, space="PSUM") as pp,
        ):
            w1 = cp.tile([2 * MOL_C, HID_C], bf16)
            nc.sync.dma_start(out=w1[:], in_=w1t[:, :])
            w2 = cp.tile([HID_C, HID_C], bf16)
            nc.sync.dma_start(out=w2[:], in_=w2t[:, :])
            w3 = cp.tile([HID_C, HID_C], bf16)
            nc.sync.dma_start(out=w3[:], in_=w3t[:, :])
            b1s = cp.tile([HID_C, 1], f32)
            nc.sync.dma_start(out=b1s[:], in_=b1[:, :])
            b3s = cp.tile([HID_C, 1], f32)
            nc.sync.dma_start(out=b3s[:], in_=b3[:, :])

            # Prime engines on the freshly-DMA'd constants so no later
            # fused-LDW matmul needs two sync waits (walrus allows one).
            prime_ps = pp.tile([HID_C, 8], f32, tag="pa")
            prime_sb = cp.tile([HID_C, 8], f32)
            nc.tensor.matmul(prime_ps[:, 0:1], w1[:], w1[:, 0:1],
                             start=True, stop=True)
            nc.tensor.matmul(prime_ps[:, 1:2], w2[:], w2[:, 0:1],
                             start=True, stop=True)
            nc.tensor.matmul(prime_ps[:, 2:3], w3[:], w3[:, 0:1],
                             start=True, stop=True)
            nc.scalar.activation(prime_sb[:, 0:1], b1s[:, 0:1], Act.Copy)
            nc.scalar.activation(prime_sb[:, 1:2], b3s[:, 0:1], Act.Copy)

            x1d = None
            for ch in range(nchunk):
                par = ch % 2
                # chunk 0: split input DMAs so the first matmuls (and hence
                # the whole ACT->DVE pipeline) start as early as possible
                nsplit = 4 if ch == 0 else 1
                if par == 0:
                    x1d = xp.tile([2 * MOL_C, C], bf16, tag="x1",
                                  name=f"x1_{ch}")
                    pr = ch // 2
                    for sp0 in range(nsplit):
                        ssl = slice(sp0 * C // nsplit, (sp0 + 1) * C // nsplit)
                        dsl = slice(pr * C + sp0 * C // nsplit,
                                    pr * C + (sp0 + 1) * C // nsplit)
                        nc.sync.dma_start(out=x1d[:, ssl], in_=x1t[:, dsl])
                x1lo = slice(par * MOL_C, (par + 1) * MOL_C)
                x2 = xp.tile([HID_C, C], bf16, tag="x2", name=f"x2_{ch}")
                for sp0 in range(nsplit):
                    ssl = slice(sp0 * C // nsplit, (sp0 + 1) * C // nsplit)
                    dsl = slice(ch * C + sp0 * C // nsplit,
                                ch * C + (sp0 + 1) * C // nsplit)
                    nc.sync.dma_start(out=x2[:, ssl], in_=x2t[:, dsl])

                atts = mp.tile([HID_C, C], bf16, tag="atts",
                               name=f"atts_{ch}")
                m2s = mp.tile([HID_C, C], bf16, tag="m2s", name=f"m2s_{ch}")
                g = sp.tile([HID_C, C], bf16, tag="g", name=f"g_{ch}")
                # weight-outer ordering: each stationary weight serves 4
                # consecutive N=512 matmuls so LDWEIGHTS amortizes.
                for half in range(NBLK // 2):
                    blks = (2 * half, 2 * half + 1)
                    pas = [pp.tile([HID_C, 1024], f32, tag="pa",
                                   name=f"pa_{ch}_{half}_{i}")
                           for i in range(2)]
                    pms = [pp.tile([HID_C, 1024], f32, tag="pm",
                                   name=f"pm_{ch}_{half}_{i}")
                           for i in range(2)]
                    for wt, wsl, xt, xsl, outs, st, sp_ in (
                        (w1, x1lo, x1d, x1lo, pas, True, False),
                        (w2, slice(None), x2, slice(None), pas, False, True),
                        (w3, slice(None), x2, slice(None), pms, True, True),
                    ):
                        for i, blk in enumerate(blks):
                            for j in range(2):
                                sl = slice(blk * 1024 + j * 512,
                                           blk * 1024 + (j + 1) * 512)
                                ps = slice(j * 512, (j + 1) * 512)
                                nc.tensor.matmul(outs[i][:, ps], wt[wsl, :],
                                                 xt[xsl, sl],
                                                 start=st, stop=sp_)
                    for i, blk in enumerate(blks):
                        bsl = slice(blk * 1024, (blk + 1) * 1024)
                        nc.scalar.activation(atts[:, bsl], pas[i][:],
                                             Act.Sigmoid, bias=b1s[:, :1],
                                             scale=1.0)
                        if need_b3 or blk < 3:
                            # staged path: ACT casts pm to bf16 SBUF (+b3),
                            # DVE multiply runs in the 2x packed mode
                            nc.scalar.activation(m2s[:, bsl], pms[i][:],
                                                 Act.Identity,
                                                 bias=b3s[:, :1], scale=1.0)
                            if need_b3 and blk == 3 and ch > 0:
                                nc.vector.tensor_tensor(
                                    out=g[:, bsl], in0=atts[:, bsl],
                                    in1=m2s[:, bsl], op=Alu.mult)
                        else:
                            # balance path (b3==0): skip the ACT cast, DVE
                            # reads the matmul PSUM directly at 1x
                            nc.vector.tensor_tensor(out=g[:, bsl],
                                                    in0=atts[:, bsl],
                                                    in1=pms[i][:],
                                                    op=Alu.mult)
                        if ch == 0 and (need_b3 or blk < 3):
                            # ramp: per-block multiplies so DVE starts early
                            nc.vector.tensor_tensor(out=g[:, bsl],
                                                    in0=atts[:, bsl],
                                                    in1=m2s[:, bsl],
                                                    op=Alu.mult)
                        elif ch > 0 and blk == 2:
                            # one merged 2x multiply for blocks 0-2
                            msl = slice(0, 3 * 1024)
                            nc.vector.tensor_tensor(out=g[:, msl],
                                                    in0=atts[:, msl],
                                                    in1=m2s[:, msl],
                                                    op=Alu.mult)

                # The host places the 4 rows of decimation group j at
                # columns j, j+DEC, j+2*DEC, j+3*DEC, so the 4-to-1
                # pair-reduce is two contiguous-half adds (bf16 2x mode).
                r2 = sp.tile([HID_C, C // 2], bf16, tag="r2", name=f"r2_{ch}")
                nc.vector.tensor_tensor(out=r2[:], in0=g[:, :C // 2],
                                        in1=g[:, C // 2:], op=Alu.add)
                r4 = sp.tile([HID_C, DEC], f32, tag="r4", name=f"r4_{ch}")
                nc.vector.tensor_tensor(out=r4[:], in0=r2[:, :DEC],
                                        in1=r2[:, DEC:], op=Alu.add)
                dtile = sp.tile([HID_C, DEC], f32, tag="dt", name=f"dt_{ch}")
                nc.vector.tensor_tensor_scan(
                    out=dtile[:], data0=r4[:], data1=r4[:], initial=0.0,
                    op0=Alu.add, op1=Alu.bypass,
                )
                nc.sync.dma_start(out=dec[:, ch * DEC:(ch + 1) * DEC],
                                  in_=dtile[:])
    nc.compile()
    return nc


def kernel(input_rep, final_rep, graph_index, lin_w, lin_b, last_w, last_b):
    global LAST_RESULTS
    import ml_dtypes
    from concourse.bass_utils import run_bass_kernel_spmd

    bf16 = ml_dtypes.bfloat16
    x1 = np.ascontiguousarray(np.asarray(input_rep, dtype=np.float32))
    x2 = np.ascontiguousarray(np.asarray(final_rep, dtype=np.float32))
    gi = np.asarray(graph_index).astype(np.int64)
    lw = np.asarray(lin_w, dtype=np.float32)
    lb = np.asarray(lin_b, dtype=np.float32)
    tw = np.asarray(last_w, dtype=np.float32)
    tb = np.asarray(last_b, dtype=np.float32)

    counts = np.bincount(gi, minlength=NUM_GRAPHS).astype(np.int64)
    pc = ((counts + PAD - 1) // PAD) * PAD          # padded per-graph rows
    row_begin = np.concatenate([[0], np.cumsum(counts)])  # src row offsets

    # per-core greedy chunk packing of whole (padded) graphs
    packing = []
    nchunk = 0
    for k in range(N_CORES):
        glo, ghi = k * GPC, (k + 1) * GPC
        pk = pc[glo:ghi]
        chunk_id = np.empty(GPC, dtype=np.int64)
        local_start = np.empty(GPC, dtype=np.int64)
        cum = 0
        ch = 0
        for i in range(GPC):
            p = pk[i]
            if cum + p > C:
                ch += 1
                cum = 0
            chunk_id[i] = ch
            local_start[i] = cum
            cum += p
        packing.append((chunk_id, local_start))
        nchunk = max(nchunk, ch + 1)
    assert nchunk <= NCHUNK_CAP, f"needs {nchunk} chunks > {NCHUNK_CAP}"
    rt = nchunk * C
    npair = (nchunk + 1) // 2

    need_b3 = bool(np.any(tb != 0.0))
    nc = _build_bass(nchunk, need_b3)

    w1t = np.zeros((2 * MOL_C, HID_C), dtype=bf16)
    w1t[:MOL_C] = lw[:, :MOL_C].T.astype(bf16)
    w1t[MOL_C:] = w1t[:MOL_C]
    w2t = np.ascontiguousarray(lw[:, MOL_C:].T).astype(bf16)
    w3t = np.ascontiguousarray(tw.T).astype(bf16)
    b1v = np.ascontiguousarray(lb.reshape(HID_C, 1))
    b3v = np.ascontiguousarray(tb.reshape(HID_C, 1))

    in_maps = []
    ext = []
    for k in range(N_CORES):
        glo, ghi = k * GPC, (k + 1) * GPC
        ck = counts[glo:ghi]
        pk = pc[glo:ghi]
        chunk_id, local_start = packing[k]

        # destination rows for real node rows
        nk = int(ck.sum())
        dst_base = chunk_id * C + local_start
        src0 = row_begin[glo]
        within = np.arange(src0, src0 + nk) - np.repeat(row_begin[glo:ghi], ck)
        dst = np.repeat(dst_base, ck) + within
        # column permutation: row L of a chunk lands at column
        # (L//PAD) + (L%PAD)*DEC so the pair-reduce reads contiguous halves
        lc = dst % C
        dst = (dst - lc) + (lc // PAD) + (lc % PAD) * DEC

        # x1: chunk pairs stacked along the partition axis
        x1t = np.zeros((2 * MOL_C, npair * C), dtype=bf16)
        dch = dst // C
        dcol = (dch // 2) * C + (dst % C)
        drow = (dch % 2) * MOL_C
        x1v = x1[src0:src0 + nk].T.astype(bf16)       # [64, nk]
        even = drow == 0
        x1t[:MOL_C, dcol[even]] = x1v[:, even]
        x1t[MOL_C:, dcol[~even]] = x1v[:, ~even]

        x2t = np.zeros((HID_C, rt), dtype=bf16)
        x2t[:, dst] = x2[src0:src0 + nk].T.astype(bf16)

        in_maps.append({
            "x1t": x1t, "x2t": x2t, "w1t": w1t, "w2t": w2t, "w3t": w3t,
            "b1": b1v, "b3": b3v,
        })
        ext.append((ck, pk, chunk_id, local_start))

    res = run_bass_kernel_spmd(nc, in_maps, core_ids=list(range(N_CORES)))
    LAST_RESULTS = res

    # pad-row gated value (zero when biases are zero)
    pad_g = (1.0 / (1.0 + np.exp(-lb))) * tb          # [HID_C]

    out = np.empty((NUM_GRAPHS, HID_C), dtype=np.float32)
    for k in range(N_CORES):
        deck = np.asarray(res.results[k]["dec"])      # [HID_C, nchunk*DEC]
        ck, pk, chunk_id, local_start = ext[k]
        end_col = chunk_id * DEC + (local_start + pk) // PAD - 1
        start_col = chunk_id * DEC + local_start // PAD - 1
        e = deck[:, end_col]                          # [HID_C, GPC]
        s = deck[:, start_col]
        s[:, local_start == 0] = 0.0
        o = (e - s).T                                 # [GPC, HID_C]
        o -= (pk - ck)[:, None].astype(np.float32) * pad_g[None, :]
        o[ck == 0] = 0.0
        out[k * GPC:(k + 1) * GPC] = o
    return out


# revision 34
# speedup vs baseline: 1.6083x; 1.0343x over previous
"""AttentionPooling kernel for 8 TRN2 NeuronCores.

Strategy (feature-major, scan-based segment sum):
  - Host shards nodes across 8 cores at graph boundaries (graph_index is
    sorted), pads each graph's rows to a multiple of PAD, packs whole graphs
    into fixed-size chunks of C rows (chunk tails zero-padded), and
    pre-transposes inputs to feature-major bf16 layout. x1 chunk PAIRS are
    stacked on the partition axis ([0:64] even chunk, [64:128] odd chunk) so
    x1 DMAs use all 128 partitions; the matmuls address PE row groups.
  - Device (SPMD, identical program on 8 cores):
      att.T  = sigmoid(W1 @ x1.T + W2 @ x2.T + b1)      (PE + ACT, bf16 in)
      m2.T   = W3 @ x2.T + b3                           (PE + ACT)
      g.T    = att.T * m2.T                             (DVE, bf16 2x mode)
      r4     = 4-column pair-reduce of g.T              (DVE, fp32 out)
      dec    = prefix-scan of r4 along rows, per chunk  (DVE scan, fp32)
    Weights stay stationary across 4 consecutive matmuls (weight-outer
    ordering) so LDWEIGHTS amortizes. Because every graph starts/ends on a
    multiple of PAD rows, every graph boundary's cumulative sum is present
    in the decimated output.
  - Host extracts per-graph sums as differences of decimated scan values,
    corrects for in-graph padding rows (nonzero only if biases nonzero),
    zeroes empty graphs, and concatenates the per-core graph ranges.
"""

import numpy as np

NUM_GRAPHS = 50000
N_NODES = 1_000_000
MOL_C = 64
HID_C = 128
N_CORES = 8
GPC = NUM_GRAPHS // N_CORES          # graphs per core
PAD = 4                              # pad each graph's rows to multiple of PAD
C = 4096                             # rows per device chunk
DEC = C // PAD                       # decimated cols per chunk
NCHUNK_CAP = 40                      # sanity cap on chunks per core
NBLK = C // 1024                     # psum blocks per chunk

LAST_RESULTS = None                  # stash for profiling from test harness


def _build_bass(nchunk: int, need_b3: bool):
    import concourse.bacc as bacc
    import concourse.tile as tile
    from concourse import mybir

    f32 = mybir.dt.float32
    bf16 = mybir.dt.bfloat16
    nc = bacc.Bacc()

    rt = nchunk * C
    npair = (nchunk + 1) // 2
    x1t = nc.dram_tensor("x1t", [2 * MOL_C, npair * C], bf16,
                         kind="ExternalInput")
    x2t = nc.dram_tensor("x2t", [HID_C, rt], bf16, kind="ExternalInput")
    w1t = nc.dram_tensor("w1t", [2 * MOL_C, HID_C], bf16, kind="ExternalInput")
    w2t = nc.dram_tensor("w2t", [HID_C, HID_C], bf16, kind="ExternalInput")
    w3t = nc.dram_tensor("w3t", [HID_C, HID_C], bf16, kind="ExternalInput")
    b1 = nc.dram_tensor("b1", [HID_C, 1], f32, kind="ExternalInput")
    b3 = nc.dram_tensor("b3", [HID_C, 1], f32, kind="ExternalInput")
    dec = nc.dram_tensor("dec", [HID_C, nchunk * DEC], f32,
                         kind="ExternalOutput")

    Act = mybir.ActivationFunctionType
    Alu = mybir.AluOpType

    with tile.TileContext(nc) as tc:
        with (
            tc.tile_pool(name="const", bufs=1) as cp,
            tc.tile_pool(name="xin", bufs=3) as xp,
            tc.tile_pool(name="mid", bufs=2) as mp,
            tc.tile_pool(name="scan", bufs=2) as sp,
            tc.tile_pool(name="psum", bufs=2, space="PSUM") as pp,
        ):
            w1 = cp.tile([2 * MOL_C, HID_C], bf16)
            nc.sync.dma_start(out=w1[:], in_=w1t[:, :])
            w2 = cp.tile([HID_C, HID_C], bf16)
            nc.sync.dma_start(out=w2[:], in_=w2t[:, :])
            w3 = cp.tile([HID_C, HID_C], bf16)
            nc.sync.dma_start(out=w3[:], in_=w3t[:, :])
            b1s = cp.tile([HID_C, 1], f32)
            nc.sync.dma_start(out=b1s[:], in_=b1[:, :])
            b3s = cp.tile([HID_C, 1], f32)
            nc.sync.dma_start(out=b3s[:], in_=b3[:, :])

            # Prime engines on the freshly-DMA'd constants so no later
            # fused-LDW matmul needs two sync waits (walrus allows one).
            prime_ps = pp.tile([HID_C, 8], f32, tag="pa")
            prime_sb = cp.tile([HID_C, 8], f32)
            nc.tensor.matmul(prime_ps[:, 0:1], w1[:], w1[:, 0:1],
                             start=True, stop=True)
            nc.tensor.matmul(prime_ps[:, 1:2], w2[:], w2[:, 0:1],
                             start=True, stop=True)
            nc.tensor.matmul(prime_ps[:, 2:3], w3[:], w3[:, 0:1],
                             start=True, stop=True)
            nc.scalar.activation(prime_sb[:, 0:1], b1s[:, 0:1], Act.Copy)
            nc.scalar.activation(prime_sb[:, 1:2], b3s[:, 0:1], Act.Copy)

            x1d = None
            for ch in range(nchunk):
                par = ch % 2
                # chunk 0: split input DMAs so the first matmuls (and hence
                # the whole ACT->DVE pipeline) start as early as possible
                nsplit = 4 if ch == 0 else 1
                if par == 0:
                    x1d = xp.tile([2 * MOL_C, C], bf16, tag="x1",
                                  name=f"x1_{ch}")
                    pr = ch // 2
                    for sp0 in range(nsplit):
                        ssl = slice(sp0 * C // nsplit, (sp0 + 1) * C // nsplit)
                        dsl = slice(pr * C + sp0 * C // nsplit,
                                    pr * C + (sp0 + 1) * C // nsplit)
                        nc.sync.dma_start(out=x1d[:, ssl], in_=x1t[:, dsl])
                x1lo = slice(par * MOL_C, (par + 1) * MOL_C)
                x2 = xp.tile([HID_C, C], bf16, tag="x2", name=f"x2_{ch}")
                for sp0 in range(nsplit):
                    ssl = slice(sp0 * C // nsplit, (sp0 + 1) * C // nsplit)
                    dsl = slice(ch * C + sp0 * C // nsplit,
                                ch * C + (sp0 + 1) * C // nsplit)
                    nc.sync.dma_start(out=x2[:, ssl], in_=x2t[:, dsl])

                atts = mp.tile([HID_C, C], bf16, tag="atts",
                               name=f"atts_{ch}")
                m2s = mp.tile([HID_C, C], bf16, tag="m2s", name=f"m2s_{ch}")
                g = sp.tile([HID_C, C], bf16, tag="g", name=f"g_{ch}")
                # weight-outer ordering: each stationary weight serves 4
                # consecutive N=512 matmuls so LDWEIGHTS amortizes.
                for half in range(NBLK // 2):
                    blks = (2 * half, 2 * half + 1)
                    pas = [pp.tile([HID_C, 1024], f32, tag="pa",
                                   name=f"pa_{ch}_{half}_{i}")
                           for i in range(2)]
                    pms = [pp.tile([HID_C, 1024], f32, tag="pm",
                                   name=f"pm_{ch}_{half}_{i}")
                           for i in range(2)]
                    def mm_pass(wt, wsl, xt, xsl, out, blk, st, sp_):
                        for j in range(2):
                            sl = slice(blk * 1024 + j * 512,
                                       blk * 1024 + (j + 1) * 512)
                            ps = slice(j * 512, (j + 1) * 512)
                            nc.tensor.matmul(out[:, ps], wt[wsl, :],
                                             xt[xsl, sl], start=st, stop=sp_)

                    full = slice(None)
                    # w1 over both blocks (stationary x4), then per block
                    # w2 then w3 so the m2 PSUM is ready soon after att's —
                    # keeps ACT from stalling between sigmoid and identity
                    for i, blk in enumerate(blks):
                        mm_pass(w1, x1lo, x1d, x1lo, pas[i], blk, True, False)
                    for i, blk in enumerate(blks):
                        mm_pass(w2, full, x2, full, pas[i], blk, False, True)
                        mm_pass(w3, full, x2, full, pms[i], blk, True, True)
                    for i, blk in enumerate(blks):
                        bsl = slice(blk * 1024, (blk + 1) * 1024)
                        nc.scalar.activation(atts[:, bsl], pas[i][:],
                                             Act.Sigmoid, bias=b1s[:, :1],
                                             scale=1.0)
                        if need_b3 or blk < 3:
                            # staged path: ACT casts pm to bf16 SBUF (+b3),
                            # DVE multiply runs in the 2x packed mode
                            nc.scalar.activation(m2s[:, bsl], pms[i][:],
                                                 Act.Identity,
                                                 bias=b3s[:, :1], scale=1.0)
                            if need_b3 and blk == 3 and ch > 0:
                                nc.vector.tensor_tensor(
                                    out=g[:, bsl], in0=atts[:, bsl],
                                    in1=m2s[:, bsl], op=Alu.mult)
                        else:
                            # balance path (b3==0): skip the ACT cast, DVE
                            # reads the matmul PSUM directly at 1x
                            nc.vector.tensor_tensor(out=g[:, bsl],
                                                    in0=atts[:, bsl],
                                                    in1=pms[i][:],
                                                    op=Alu.mult)
                        if ch == 0 and (need_b3 or blk < 3):
                            # ramp: per-block multiplies so DVE starts early
                            nc.vector.tensor_tensor(out=g[:, bsl],
                                                    in0=atts[:, bsl],
                                                    in1=m2s[:, bsl],
                                                    op=Alu.mult)
                        elif ch > 0 and blk == 2:
                            # one merged 2x multiply for blocks 0-2
                            msl = slice(0, 3 * 1024)
                            nc.vector.tensor_tensor(out=g[:, msl],
                                                    in0=atts[:, msl],
                                                    in1=m2s[:, msl],
                                                    op=Alu.mult)

                # The host places the 4 rows of decimation group j at
                # columns j, j+DEC, j+2*DEC, j+3*DEC, so the 4-to-1
                # pair-reduce is two contiguous-half adds (bf16 2x mode).
                r2 = sp.tile([HID_C, C // 2], bf16, tag="r2", name=f"r2_{ch}")
                nc.vector.tensor_tensor(out=r2[:], in0=g[:, :C // 2],
                                        in1=g[:, C // 2:], op=Alu.add)
                r4 = sp.tile([HID_C, DEC], bf16, tag="r4", name=f"r4_{ch}")
                nc.vector.tensor_tensor(out=r4[:], in0=r2[:, :DEC],
                                        in1=r2[:, DEC:], op=Alu.add)
                dtile = sp.tile([HID_C, DEC], f32, tag="dt", name=f"dt_{ch}")
                nc.vector.tensor_tensor_scan(
                    out=dtile[:], data0=r4[:], data1=r4[:], initial=0.0,
                    op0=Alu.add, op1=Alu.bypass,
                )
                nc.sync.dma_start(out=dec[:, ch * DEC:(ch + 1) * DEC],
                                  in_=dtile[:])
    nc.compile()
    return nc


def kernel(input_rep, final_rep, graph_index, lin_w, lin_b, last_w, last_b):
    global LAST_RESULTS
    import ml_dtypes
    from concourse.bass_utils import run_bass_kernel_spmd

    bf16 = ml_dtypes.bfloat16
    x1 = np.ascontiguousarray(np.asarray(input_rep, dtype=np.float32))
    x2 = np.ascontiguousarray(np.asarray(final_rep, dtype=np.float32))
    gi = np.asarray(graph_index).astype(np.int64)
    lw = np.asarray(lin_w, dtype=np.float32)
    lb = np.asarray(lin_b, dtype=np.float32)
    tw = np.asarray(last_w, dtype=np.float32)
    tb = np.asarray(last_b, dtype=np.float32)

    counts = np.bincount(gi, minlength=NUM_GRAPHS).astype(np.int64)
    pc = ((counts + PAD - 1) // PAD) * PAD          # padded per-graph rows
    row_begin = np.concatenate([[0], np.cumsum(counts)])  # src row offsets

    # per-core greedy chunk packing of whole (padded) graphs
    packing = []
    nchunk = 0
    for k in range(N_CORES):
        glo, ghi = k * GPC, (k + 1) * GPC
        pk = pc[glo:ghi]
        chunk_id = np.empty(GPC, dtype=np.int64)
        local_start = np.empty(GPC, dtype=np.int64)
        cum = 0
        ch = 0
        for i in range(GPC):
            p = pk[i]
            if cum + p > C:
                ch += 1
                cum = 0
            chunk_id[i] = ch
            local_start[i] = cum
            cum += p
        packing.append((chunk_id, local_start))
        nchunk = max(nchunk, ch + 1)
    assert nchunk <= NCHUNK_CAP, f"needs {nchunk} chunks > {NCHUNK_CAP}"
    rt = nchunk * C
    npair = (nchunk + 1) // 2

    need_b3 = bool(np.any(tb != 0.0))
    nc = _build_bass(nchunk, need_b3)

    w1t = np.zeros((2 * MOL_C, HID_C), dtype=bf16)
    w1t[:MOL_C] = lw[:, :MOL_C].T.astype(bf16)
    w1t[MOL_C:] = w1t[:MOL_C]
    w2t = np.ascontiguousarray(lw[:, MOL_C:].T).astype(bf16)
    w3t = np.ascontiguousarray(tw.T).astype(bf16)
    b1v = np.ascontiguousarray(lb.reshape(HID_C, 1))
    b3v = np.ascontiguousarray(tb.reshape(HID_C, 1))

    in_maps = []
    ext = []
    for k in range(N_CORES):
        glo, ghi = k * GPC, (k + 1) * GPC
        ck = counts[glo:ghi]
        pk = pc[glo:ghi]
        chunk_id, local_start = packing[k]

        # destination rows for real node rows
        nk = int(ck.sum())
        dst_base = chunk_id * C + local_start
        src0 = row_begin[glo]
        within = np.arange(src0, src0 + nk) - np.repeat(row_begin[glo:ghi], ck)
        dst = np.repeat(dst_base, ck) + within
        # column permutation: row L of a chunk lands at column
        # (L//PAD) + (L%PAD)*DEC so the pair-reduce reads contiguous halves
        lc = dst % C
        dst = (dst - lc) + (lc // PAD) + (lc % PAD) * DEC

        # x1: chunk pairs stacked along the partition axis
        x1t = np.zeros((2 * MOL_C, npair * C), dtype=bf16)
        dch = dst // C
        dcol = (dch // 2) * C + (dst % C)
        drow = (dch % 2) * MOL_C
        x1v = x1[src0:src0 + nk].T.astype(bf16)       # [64, nk]
        even = drow == 0
        x1t[:MOL_C, dcol[even]] = x1v[:, even]
        x1t[MOL_C:, dcol[~even]] = x1v[:, ~even]

        x2t = np.zeros((HID_C, rt), dtype=bf16)
        x2t[:, dst] = x2[src0:src0 + nk].T.astype(bf16)

        in_maps.append({
            "x1t": x1t, "x2t": x2t, "w1t": w1t, "w2t": w2t, "w3t": w3t,
            "b1": b1v, "b3": b3v,
        })
        ext.append((ck, pk, chunk_id, local_start))

    res = run_bass_kernel_spmd(nc, in_maps, core_ids=list(range(N_CORES)))
    LAST_RESULTS = res

    # pad-row gated value (zero when biases are zero)
    pad_g = (1.0 / (1.0 + np.exp(-lb))) * tb          # [HID_C]

    out = np.empty((NUM_GRAPHS, HID_C), dtype=np.float32)
    for k in range(N_CORES):
        deck = np.asarray(res.results[k]["dec"])      # [HID_C, nchunk*DEC]
        ck, pk, chunk_id, local_start = ext[k]
        end_col = chunk_id * DEC + (local_start + pk) // PAD - 1
        start_col = chunk_id * DEC + local_start // PAD - 1
        e = deck[:, end_col]                          # [HID_C, GPC]
        s = deck[:, start_col]
        s[:, local_start == 0] = 0.0
        o = (e - s).T                                 # [GPC, HID_C]
        o -= (pk - ck)[:, None].astype(np.float32) * pad_g[None, :]
        o[ck == 0] = 0.0
        out[k * GPC:(k + 1) * GPC] = o
    return out


# revision 36
# speedup vs baseline: 1.6497x; 1.0257x over previous
"""AttentionPooling kernel for 8 TRN2 NeuronCores.

Strategy (feature-major, scan-based segment sum):
  - Host shards nodes across 8 cores at graph boundaries (graph_index is
    sorted), pads each graph's rows to a multiple of PAD, packs whole graphs
    into fixed-size chunks of C rows (chunk tails zero-padded), and
    pre-transposes inputs to feature-major bf16 layout. x1 chunk PAIRS are
    stacked on the partition axis ([0:64] even chunk, [64:128] odd chunk) so
    x1 DMAs use all 128 partitions; the matmuls address PE row groups.
  - Device (SPMD, identical program on 8 cores):
      att.T  = sigmoid(W1 @ x1.T + W2 @ x2.T + b1)      (PE + ACT, bf16 in)
      m2.T   = W3 @ x2.T + b3                           (PE + ACT)
      g.T    = att.T * m2.T                             (DVE, bf16 2x mode)
      r4     = 4-column pair-reduce of g.T              (DVE, fp32 out)
      dec    = prefix-scan of r4 along rows, per chunk  (DVE scan, fp32)
    Weights stay stationary across 4 consecutive matmuls (weight-outer
    ordering) so LDWEIGHTS amortizes. Because every graph starts/ends on a
    multiple of PAD rows, every graph boundary's cumulative sum is present
    in the decimated output.
  - Host extracts per-graph sums as differences of decimated scan values,
    corrects for in-graph padding rows (nonzero only if biases nonzero),
    zeroes empty graphs, and concatenates the per-core graph ranges.
"""

import numpy as np

NUM_GRAPHS = 50000
N_NODES = 1_000_000
MOL_C = 64
HID_C = 128
N_CORES = 8
GPC = NUM_GRAPHS // N_CORES          # graphs per core
PAD = 4                              # pad each graph's rows to multiple of PAD
C = 4096                             # rows per device chunk
DEC = C // PAD                       # decimated cols per chunk
NCHUNK_CAP = 40                      # sanity cap on chunks per core
NBLK = C // 1024                     # psum blocks per chunk

LAST_RESULTS = None                  # stash for profiling from test harness


def _build_bass(nchunk: int, need_b3: bool):
    import concourse.bacc as bacc
    import concourse.tile as tile
    from concourse import mybir

    f32 = mybir.dt.float32
    bf16 = mybir.dt.bfloat16
    nc = bacc.Bacc()

    rt = nchunk * C
    npair = (nchunk + 1) // 2
    x1t = nc.dram_tensor("x1t", [2 * MOL_C, npair * C], bf16,
                         kind="ExternalInput")
    x2t = nc.dram_tensor("x2t", [HID_C, rt], bf16, kind="ExternalInput")
    w1t = nc.dram_tensor("w1t", [2 * MOL_C, HID_C], bf16, kind="ExternalInput")
    w2t = nc.dram_tensor("w2t", [HID_C, HID_C], bf16, kind="ExternalInput")
    w3t = nc.dram_tensor("w3t", [HID_C, HID_C], bf16, kind="ExternalInput")
    b1 = nc.dram_tensor("b1", [HID_C, 1], f32, kind="ExternalInput")
    b3 = nc.dram_tensor("b3", [HID_C, 1], f32, kind="ExternalInput")
    dec = nc.dram_tensor("dec", [HID_C, nchunk * DEC], f32,
                         kind="ExternalOutput")

    Act = mybir.ActivationFunctionType
    Alu = mybir.AluOpType

    with tile.TileContext(nc) as tc:
        with (
            tc.tile_pool(name="const", bufs=1) as cp,
            tc.tile_pool(name="xin", bufs=3) as xp,
            tc.tile_pool(name="mid", bufs=2) as mp,
            tc.tile_pool(name="scan", bufs=2) as sp,
            tc.tile_pool(name="psum", bufs=2, space="PSUM") as pp,
        ):
            w1 = cp.tile([2 * MOL_C, HID_C], bf16)
            nc.sync.dma_start(out=w1[:], in_=w1t[:, :])
            w2 = cp.tile([HID_C, HID_C], bf16)
            nc.sync.dma_start(out=w2[:], in_=w2t[:, :])
            w3 = cp.tile([HID_C, HID_C], bf16)
            nc.sync.dma_start(out=w3[:], in_=w3t[:, :])
            b1s = cp.tile([HID_C, 1], f32)
            nc.sync.dma_start(out=b1s[:], in_=b1[:, :])
            b3s = cp.tile([HID_C, 1], f32)
            nc.sync.dma_start(out=b3s[:], in_=b3[:, :])

            # Prime engines on the freshly-DMA'd constants so no later
            # fused-LDW matmul needs two sync waits (walrus allows one).
            prime_ps = pp.tile([HID_C, 8], f32, tag="pa")
            prime_sb = cp.tile([HID_C, 8], f32)
            nc.tensor.matmul(prime_ps[:, 0:1], w1[:], w1[:, 0:1],
                             start=True, stop=True)
            nc.tensor.matmul(prime_ps[:, 1:2], w2[:], w2[:, 0:1],
                             start=True, stop=True)
            nc.tensor.matmul(prime_ps[:, 2:3], w3[:], w3[:, 0:1],
                             start=True, stop=True)
            nc.scalar.activation(prime_sb[:, 0:1], b1s[:, 0:1], Act.Copy)
            nc.scalar.activation(prime_sb[:, 1:2], b3s[:, 0:1], Act.Copy)

            x1d = None
            for ch in range(nchunk):
                par = ch % 2
                # chunk 0: split input DMAs so the first matmuls (and hence
                # the whole ACT->DVE pipeline) start as early as possible
                nsplit = 4 if ch == 0 else 1
                if par == 0:
                    x1d = xp.tile([2 * MOL_C, C], bf16, tag="x1",
                                  name=f"x1_{ch}")
                    pr = ch // 2
                    for sp0 in range(nsplit):
                        ssl = slice(sp0 * C // nsplit, (sp0 + 1) * C // nsplit)
                        dsl = slice(pr * C + sp0 * C // nsplit,
                                    pr * C + (sp0 + 1) * C // nsplit)
                        nc.sync.dma_start(out=x1d[:, ssl], in_=x1t[:, dsl])
                x1lo = slice(par * MOL_C, (par + 1) * MOL_C)
                x2 = xp.tile([HID_C, C], bf16, tag="x2", name=f"x2_{ch}")
                for sp0 in range(nsplit):
                    ssl = slice(sp0 * C // nsplit, (sp0 + 1) * C // nsplit)
                    dsl = slice(ch * C + sp0 * C // nsplit,
                                ch * C + (sp0 + 1) * C // nsplit)
                    nc.sync.dma_start(out=x2[:, ssl], in_=x2t[:, dsl])

                atts = mp.tile([HID_C, C], bf16, tag="atts",
                               name=f"atts_{ch}")
                m2s = mp.tile([HID_C, C], bf16, tag="m2s", name=f"m2s_{ch}")
                g = sp.tile([HID_C, C], bf16, tag="g", name=f"g_{ch}")
                # weight-outer ordering: each stationary weight serves 4
                # consecutive N=512 matmuls so LDWEIGHTS amortizes.
                for half in range(NBLK // 2):
                    blks = (2 * half, 2 * half + 1)
                    pas = [pp.tile([HID_C, 1024], f32, tag="pa",
                                   name=f"pa_{ch}_{half}_{i}")
                           for i in range(2)]
                    pms = [pp.tile([HID_C, 1024], f32, tag="pm",
                                   name=f"pm_{ch}_{half}_{i}")
                           for i in range(2)]
                    def mm_pass(wt, wsl, xt, xsl, out, blk, st, sp_):
                        for j in range(2):
                            sl = slice(blk * 1024 + j * 512,
                                       blk * 1024 + (j + 1) * 512)
                            ps = slice(j * 512, (j + 1) * 512)
                            nc.tensor.matmul(out[:, ps], wt[wsl, :],
                                             xt[xsl, sl], start=st, stop=sp_)

                    full = slice(None)
                    # w1 over both blocks (stationary x4), then per block
                    # w2 then w3 so the m2 PSUM is ready soon after att's —
                    # keeps ACT from stalling between sigmoid and identity
                    for i, blk in enumerate(blks):
                        mm_pass(w1, x1lo, x1d, x1lo, pas[i], blk, True, False)
                    for i, blk in enumerate(blks):
                        mm_pass(w2, full, x2, full, pas[i], blk, False, True)
                        mm_pass(w3, full, x2, full, pms[i], blk, True, True)
                    for i, blk in enumerate(blks):
                        bsl = slice(blk * 1024, (blk + 1) * 1024)
                        nc.scalar.activation(atts[:, bsl], pas[i][:],
                                             Act.Sigmoid, bias=b1s[:, :1],
                                             scale=1.0)
                        if ch == 0 and not need_b3:
                            # ramp chunk: direct-PSUM multiplies shorten the
                            # startup dependency chain for the vector engine
                            nc.vector.tensor_tensor(out=g[:, bsl],
                                                    in0=atts[:, bsl],
                                                    in1=pms[i][:],
                                                    op=Alu.mult)
                        elif need_b3:
                            # general path: ACT adds b3 and casts to bf16
                            nc.scalar.activation(m2s[:, bsl], pms[i][:],
                                                 Act.Identity,
                                                 bias=b3s[:, :1], scale=1.0)
                            if ch == 0 or blk == 3:
                                nc.vector.tensor_tensor(
                                    out=g[:, bsl], in0=atts[:, bsl],
                                    in1=m2s[:, bsl], op=Alu.mult)
                            elif blk == 2:
                                msl = slice(0, 3 * 1024)
                                nc.vector.tensor_tensor(
                                    out=g[:, msl], in0=atts[:, msl],
                                    in1=m2s[:, msl], op=Alu.mult)
                        elif blk < 3:
                            # staged path: ACT casts pm to bf16 SBUF, DVE
                            # multiply runs in the 2x packed mode
                            nc.scalar.activation(m2s[:, bsl], pms[i][:],
                                                 Act.Identity,
                                                 bias=b3s[:, :1], scale=1.0)
                            if blk == 2:
                                # one merged 2x multiply for blocks 0-2
                                msl = slice(0, 3 * 1024)
                                nc.vector.tensor_tensor(
                                    out=g[:, msl], in0=atts[:, msl],
                                    in1=m2s[:, msl], op=Alu.mult)
                        else:
                            # balance path (b3==0): skip the ACT cast, DVE
                            # reads the matmul PSUM directly at 1x
                            nc.vector.tensor_tensor(out=g[:, bsl],
                                                    in0=atts[:, bsl],
                                                    in1=pms[i][:],
                                                    op=Alu.mult)

                # The host places the 4 rows of decimation group j at
                # columns j, j+DEC, j+2*DEC, j+3*DEC, so the 4-to-1
                # pair-reduce is two contiguous-half adds (bf16 2x mode).
                r2 = sp.tile([HID_C, C // 2], bf16, tag="r2", name=f"r2_{ch}")
                nc.vector.tensor_tensor(out=r2[:], in0=g[:, :C // 2],
                                        in1=g[:, C // 2:], op=Alu.add)
                r4 = sp.tile([HID_C, DEC], bf16, tag="r4", name=f"r4_{ch}")
                nc.vector.tensor_tensor(out=r4[:], in0=r2[:, :DEC],
                                        in1=r2[:, DEC:], op=Alu.add)
                dtile = sp.tile([HID_C, DEC], f32, tag="dt", name=f"dt_{ch}")
                nc.vector.tensor_tensor_scan(
                    out=dtile[:], data0=r4[:], data1=r4[:], initial=0.0,
                    op0=Alu.add, op1=Alu.bypass,
                )
                nc.sync.dma_start(out=dec[:, ch * DEC:(ch + 1) * DEC],
                                  in_=dtile[:])
    nc.compile()
    return nc


def kernel(input_rep, final_rep, graph_index, lin_w, lin_b, last_w, last_b):
    global LAST_RESULTS
    import ml_dtypes
    from concourse.bass_utils import run_bass_kernel_spmd

    bf16 = ml_dtypes.bfloat16
    x1 = np.ascontiguousarray(np.asarray(input_rep, dtype=np.float32))
    x2 = np.ascontiguousarray(np.asarray(final_rep, dtype=np.float32))
    gi = np.asarray(graph_index).astype(np.int64)
    lw = np.asarray(lin_w, dtype=np.float32)
    lb = np.asarray(lin_b, dtype=np.float32)
    tw = np.asarray(last_w, dtype=np.float32)
    tb = np.asarray(last_b, dtype=np.float32)

    counts = np.bincount(gi, minlength=NUM_GRAPHS).astype(np.int64)
    pc = ((counts + PAD - 1) // PAD) * PAD          # padded per-graph rows
    row_begin = np.concatenate([[0], np.cumsum(counts)])  # src row offsets

    # per-core greedy chunk packing of whole (padded) graphs
    packing = []
    nchunk = 0
    for k in range(N_CORES):
        glo, ghi = k * GPC, (k + 1) * GPC
        pk = pc[glo:ghi]
        chunk_id = np.empty(GPC, dtype=np.int64)
        local_start = np.empty(GPC, dtype=np.int64)
        cum = 0
        ch = 0
        for i in range(GPC):
            p = pk[i]
            if cum + p > C:
                ch += 1
                cum = 0
            chunk_id[i] = ch
            local_start[i] = cum
            cum += p
        packing.append((chunk_id, local_start))
        nchunk = max(nchunk, ch + 1)
    assert nchunk <= NCHUNK_CAP, f"needs {nchunk} chunks > {NCHUNK_CAP}"
    rt = nchunk * C
    npair = (nchunk + 1) // 2

    need_b3 = bool(np.any(tb != 0.0))
    nc = _build_bass(nchunk, need_b3)

    w1t = np.zeros((2 * MOL_C, HID_C), dtype=bf16)
    w1t[:MOL_C] = lw[:, :MOL_C].T.astype(bf16)
    w1t[MOL_C:] = w1t[:MOL_C]
    w2t = np.ascontiguousarray(lw[:, MOL_C:].T).astype(bf16)
    w3t = np.ascontiguousarray(tw.T).astype(bf16)
    b1v = np.ascontiguousarray(lb.reshape(HID_C, 1))
    b3v = np.ascontiguousarray(tb.reshape(HID_C, 1))

    in_maps = []
    ext = []
    for k in range(N_CORES):
        glo, ghi = k * GPC, (k + 1) * GPC
        ck = counts[glo:ghi]
        pk = pc[glo:ghi]
        chunk_id, local_start = packing[k]

        # destination rows for real node rows
        nk = int(ck.sum())
        dst_base = chunk_id * C + local_start
        src0 = row_begin[glo]
        within = np.arange(src0, src0 + nk) - np.repeat(row_begin[glo:ghi], ck)
        dst = np.repeat(dst_base, ck) + within
        # column permutation: row L of a chunk lands at column
        # (L//PAD) + (L%PAD)*DEC so the pair-reduce reads contiguous halves
        lc = dst % C
        dst = (dst - lc) + (lc // PAD) + (lc % PAD) * DEC

        # x1: chunk pairs stacked along the partition axis
        x1t = np.zeros((2 * MOL_C, npair * C), dtype=bf16)
        dch = dst // C
        dcol = (dch // 2) * C + (dst % C)
        drow = (dch % 2) * MOL_C
        x1v = x1[src0:src0 + nk].T.astype(bf16)       # [64, nk]
        even = drow == 0
        x1t[:MOL_C, dcol[even]] = x1v[:, even]
        x1t[MOL_C:, dcol[~even]] = x1v[:, ~even]

        x2t = np.zeros((HID_C, rt), dtype=bf16)
        x2t[:, dst] = x2[src0:src0 + nk].T.astype(bf16)

        in_maps.append({
            "x1t": x1t, "x2t": x2t, "w1t": w1t, "w2t": w2t, "w3t": w3t,
            "b1": b1v, "b3": b3v,
        })
        ext.append((ck, pk, chunk_id, local_start))

    res = run_bass_kernel_spmd(nc, in_maps, core_ids=list(range(N_CORES)))
    LAST_RESULTS = res

    # pad-row gated value (zero when biases are zero)
    pad_g = (1.0 / (1.0 + np.exp(-lb))) * tb          # [HID_C]

    out = np.empty((NUM_GRAPHS, HID_C), dtype=np.float32)
    for k in range(N_CORES):
        deck = np.asarray(res.results[k]["dec"])      # [HID_C, nchunk*DEC]
        ck, pk, chunk_id, local_start = ext[k]
        end_col = chunk_id * DEC + (local_start + pk) // PAD - 1
        start_col = chunk_id * DEC + local_start // PAD - 1
        e = deck[:, end_col]                          # [HID_C, GPC]
        s = deck[:, start_col]
        s[:, local_start == 0] = 0.0
        o = (e - s).T                                 # [GPC, HID_C]
        o -= (pk - ck)[:, None].astype(np.float32) * pad_g[None, :]
        o[ck == 0] = 0.0
        out[k * GPC:(k + 1) * GPC] = o
    return out


# revision 38
# speedup vs baseline: 1.6550x; 1.0032x over previous
"""AttentionPooling kernel for 8 TRN2 NeuronCores.

Strategy (feature-major, scan-based segment sum):
  - Host shards nodes across 8 cores at graph boundaries (graph_index is
    sorted), pads each graph's rows to a multiple of PAD, packs whole graphs
    into fixed-size chunks of C rows (chunk tails zero-padded), and
    pre-transposes inputs to feature-major bf16 layout. x1 chunk PAIRS are
    stacked on the partition axis ([0:64] even chunk, [64:128] odd chunk) so
    x1 DMAs use all 128 partitions; the matmuls address PE row groups.
  - Device (SPMD, identical program on 8 cores):
      att.T  = sigmoid(W1 @ x1.T + W2 @ x2.T + b1)      (PE + ACT, bf16 in)
      m2.T   = W3 @ x2.T + b3                           (PE + ACT)
      g.T    = att.T * m2.T                             (DVE, bf16 2x mode)
      r4     = 4-column pair-reduce of g.T              (DVE, fp32 out)
      dec    = prefix-scan of r4 along rows, per chunk  (DVE scan, fp32)
    Weights stay stationary across 4 consecutive matmuls (weight-outer
    ordering) so LDWEIGHTS amortizes. Because every graph starts/ends on a
    multiple of PAD rows, every graph boundary's cumulative sum is present
    in the decimated output.
  - Host extracts per-graph sums as differences of decimated scan values,
    corrects for in-graph padding rows (nonzero only if biases nonzero),
    zeroes empty graphs, and concatenates the per-core graph ranges.
"""

import numpy as np

NUM_GRAPHS = 50000
N_NODES = 1_000_000
MOL_C = 64
HID_C = 128
N_CORES = 8
GPC = NUM_GRAPHS // N_CORES          # graphs per core
PAD = 4                              # pad each graph's rows to multiple of PAD
C = 4096                             # rows per device chunk
DEC = C // PAD                       # decimated cols per chunk
NCHUNK_CAP = 40                      # sanity cap on chunks per core
NBLK = C // 1024                     # psum blocks per chunk

LAST_RESULTS = None                  # stash for profiling from test harness


def _build_bass(nchunk: int, need_b3: bool):
    import concourse.bacc as bacc
    import concourse.tile as tile
    from concourse import mybir

    f32 = mybir.dt.float32
    bf16 = mybir.dt.bfloat16
    nc = bacc.Bacc()

    rt = nchunk * C
    npair = (nchunk + 1) // 2
    x1t = nc.dram_tensor("x1t", [2 * MOL_C, npair * C], bf16,
                         kind="ExternalInput")
    x2t = nc.dram_tensor("x2t", [HID_C, rt], bf16, kind="ExternalInput")
    w1t = nc.dram_tensor("w1t", [2 * MOL_C, HID_C], bf16, kind="ExternalInput")
    w2t = nc.dram_tensor("w2t", [HID_C, HID_C], bf16, kind="ExternalInput")
    w3t = nc.dram_tensor("w3t", [HID_C, HID_C], bf16, kind="ExternalInput")
    b1 = nc.dram_tensor("b1", [HID_C, 1], f32, kind="ExternalInput")
    b3 = nc.dram_tensor("b3", [HID_C, 1], f32, kind="ExternalInput")
    dec = nc.dram_tensor("dec", [HID_C, nchunk * DEC], f32,
                         kind="ExternalOutput")

    Act = mybir.ActivationFunctionType
    Alu = mybir.AluOpType

    with tile.TileContext(nc) as tc:
        with (
            tc.tile_pool(name="const", bufs=1) as cp,
            tc.tile_pool(name="xin", bufs=3) as xp,
            tc.tile_pool(name="mid", bufs=2) as mp,
            tc.tile_pool(name="att3", bufs=3) as ap3,
            tc.tile_pool(name="gpool", bufs=1) as gp1,
            tc.tile_pool(name="scan", bufs=2) as sp,
            tc.tile_pool(name="psum", bufs=2, space="PSUM") as pp,
        ):
            w1 = cp.tile([2 * MOL_C, HID_C], bf16)
            nc.sync.dma_start(out=w1[:], in_=w1t[:, :])
            w2 = cp.tile([HID_C, HID_C], bf16)
            nc.sync.dma_start(out=w2[:], in_=w2t[:, :])
            w3 = cp.tile([HID_C, HID_C], bf16)
            nc.sync.dma_start(out=w3[:], in_=w3t[:, :])
            b1s = cp.tile([HID_C, 1], f32)
            nc.sync.dma_start(out=b1s[:], in_=b1[:, :])
            b3s = cp.tile([HID_C, 1], f32)
            nc.sync.dma_start(out=b3s[:], in_=b3[:, :])

            # Prime engines on the freshly-DMA'd constants so no later
            # fused-LDW matmul needs two sync waits (walrus allows one).
            prime_ps = pp.tile([HID_C, 8], f32, tag="pa")
            prime_sb = cp.tile([HID_C, 8], f32)
            nc.tensor.matmul(prime_ps[:, 0:1], w1[:], w1[:, 0:1],
                             start=True, stop=True)
            nc.tensor.matmul(prime_ps[:, 1:2], w2[:], w2[:, 0:1],
                             start=True, stop=True)
            nc.tensor.matmul(prime_ps[:, 2:3], w3[:], w3[:, 0:1],
                             start=True, stop=True)
            nc.scalar.activation(prime_sb[:, 0:1], b1s[:, 0:1], Act.Copy)
            nc.scalar.activation(prime_sb[:, 1:2], b3s[:, 0:1], Act.Copy)

            x1d = None
            for ch in range(nchunk):
                par = ch % 2
                # chunk 0: split input DMAs so the first matmuls (and hence
                # the whole ACT->DVE pipeline) start as early as possible
                nsplit = 4 if ch == 0 else 1
                if par == 0:
                    x1d = xp.tile([2 * MOL_C, C], bf16, tag="x1",
                                  name=f"x1_{ch}")
                    pr = ch // 2
                    for sp0 in range(nsplit):
                        ssl = slice(sp0 * C // nsplit, (sp0 + 1) * C // nsplit)
                        dsl = slice(pr * C + sp0 * C // nsplit,
                                    pr * C + (sp0 + 1) * C // nsplit)
                        nc.sync.dma_start(out=x1d[:, ssl], in_=x1t[:, dsl])
                x1lo = slice(par * MOL_C, (par + 1) * MOL_C)
                x2 = xp.tile([HID_C, C], bf16, tag="x2", name=f"x2_{ch}")
                for sp0 in range(nsplit):
                    ssl = slice(sp0 * C // nsplit, (sp0 + 1) * C // nsplit)
                    dsl = slice(ch * C + sp0 * C // nsplit,
                                ch * C + (sp0 + 1) * C // nsplit)
                    nc.sync.dma_start(out=x2[:, ssl], in_=x2t[:, dsl])

                atts = ap3.tile([HID_C, C], bf16, tag="atts",
                                name=f"atts_{ch}")
                m2s = mp.tile([HID_C, C], bf16, tag="m2s", name=f"m2s_{ch}")
                g = gp1.tile([HID_C, C], bf16, tag="g", name=f"g_{ch}")
                # weight-outer ordering: each stationary weight serves 4
                # consecutive N=512 matmuls so LDWEIGHTS amortizes.
                for half in range(NBLK // 2):
                    blks = (2 * half, 2 * half + 1)
                    pas = [pp.tile([HID_C, 1024], f32, tag="pa",
                                   name=f"pa_{ch}_{half}_{i}")
                           for i in range(2)]
                    pms = [pp.tile([HID_C, 1024], f32, tag="pm",
                                   name=f"pm_{ch}_{half}_{i}")
                           for i in range(2)]
                    def mm_pass(wt, wsl, xt, xsl, out, blk, st, sp_):
                        for j in range(2):
                            sl = slice(blk * 1024 + j * 512,
                                       blk * 1024 + (j + 1) * 512)
                            ps = slice(j * 512, (j + 1) * 512)
                            nc.tensor.matmul(out[:, ps], wt[wsl, :],
                                             xt[xsl, sl], start=st, stop=sp_)

                    full = slice(None)
                    # w1 over both blocks (stationary x4), then per block
                    # w2 then w3 so the m2 PSUM is ready soon after att's —
                    # keeps ACT from stalling between sigmoid and identity
                    for i, blk in enumerate(blks):
                        mm_pass(w1, x1lo, x1d, x1lo, pas[i], blk, True, False)
                    for i, blk in enumerate(blks):
                        mm_pass(w2, full, x2, full, pas[i], blk, False, True)
                        mm_pass(w3, full, x2, full, pms[i], blk, True, True)
                    for i, blk in enumerate(blks):
                        bsl = slice(blk * 1024, (blk + 1) * 1024)
                        nc.scalar.activation(atts[:, bsl], pas[i][:],
                                             Act.Sigmoid, bias=b1s[:, :1],
                                             scale=1.0)
                        if ch == 0 and not need_b3:
                            # ramp chunk: direct-PSUM multiplies shorten the
                            # startup dependency chain for the vector engine
                            nc.vector.tensor_tensor(out=g[:, bsl],
                                                    in0=atts[:, bsl],
                                                    in1=pms[i][:],
                                                    op=Alu.mult)
                        elif need_b3:
                            # general path: ACT adds b3 and casts to bf16
                            nc.scalar.activation(m2s[:, bsl], pms[i][:],
                                                 Act.Identity,
                                                 bias=b3s[:, :1], scale=1.0)
                            if ch == 0 or blk == 3:
                                nc.vector.tensor_tensor(
                                    out=g[:, bsl], in0=atts[:, bsl],
                                    in1=m2s[:, bsl], op=Alu.mult)
                            elif blk == 2:
                                msl = slice(0, 3 * 1024)
                                nc.vector.tensor_tensor(
                                    out=g[:, msl], in0=atts[:, msl],
                                    in1=m2s[:, msl], op=Alu.mult)
                        elif blk < 3:
                            # staged path: ACT casts pm to bf16 SBUF, DVE
                            # multiply runs in the 2x packed mode
                            nc.scalar.activation(m2s[:, bsl], pms[i][:],
                                                 Act.Identity,
                                                 bias=b3s[:, :1], scale=1.0)
                            if blk == 2:
                                # one merged 2x multiply for blocks 0-2
                                msl = slice(0, 3 * 1024)
                                nc.vector.tensor_tensor(
                                    out=g[:, msl], in0=atts[:, msl],
                                    in1=m2s[:, msl], op=Alu.mult)
                        else:
                            # balance path (b3==0): skip the ACT cast, DVE
                            # reads the matmul PSUM directly at 1x
                            nc.vector.tensor_tensor(out=g[:, bsl],
                                                    in0=atts[:, bsl],
                                                    in1=pms[i][:],
                                                    op=Alu.mult)

                # The host places the 4 rows of decimation group j at
                # columns j, j+DEC, j+2*DEC, j+3*DEC, so the 4-to-1
                # pair-reduce is two contiguous-half adds (bf16 2x mode).
                r2 = sp.tile([HID_C, C // 2], bf16, tag="r2", name=f"r2_{ch}")
                nc.vector.tensor_tensor(out=r2[:], in0=g[:, :C // 2],
                                        in1=g[:, C // 2:], op=Alu.add)
                r4 = sp.tile([HID_C, DEC], bf16, tag="r4", name=f"r4_{ch}")
                nc.vector.tensor_tensor(out=r4[:], in0=r2[:, :DEC],
                                        in1=r2[:, DEC:], op=Alu.add)
                dtile = sp.tile([HID_C, DEC], f32, tag="dt", name=f"dt_{ch}")
                nc.vector.tensor_tensor_scan(
                    out=dtile[:], data0=r4[:], data1=r4[:], initial=0.0,
                    op0=Alu.add, op1=Alu.bypass,
                )
                nc.sync.dma_start(out=dec[:, ch * DEC:(ch + 1) * DEC],
                                  in_=dtile[:])
    nc.compile()
    return nc


def kernel(input_rep, final_rep, graph_index, lin_w, lin_b, last_w, last_b):
    global LAST_RESULTS
    import ml_dtypes
    from concourse.bass_utils import run_bass_kernel_spmd

    bf16 = ml_dtypes.bfloat16
    x1 = np.ascontiguousarray(np.asarray(input_rep, dtype=np.float32))
    x2 = np.ascontiguousarray(np.asarray(final_rep, dtype=np.float32))
    gi = np.asarray(graph_index).astype(np.int64)
    lw = np.asarray(lin_w, dtype=np.float32)
    lb = np.asarray(lin_b, dtype=np.float32)
    tw = np.asarray(last_w, dtype=np.float32)
    tb = np.asarray(last_b, dtype=np.float32)

    counts = np.bincount(gi, minlength=NUM_GRAPHS).astype(np.int64)
    pc = ((counts + PAD - 1) // PAD) * PAD          # padded per-graph rows
    row_begin = np.concatenate([[0], np.cumsum(counts)])  # src row offsets

    # per-core greedy chunk packing of whole (padded) graphs
    packing = []
    nchunk = 0
    for k in range(N_CORES):
        glo, ghi = k * GPC, (k + 1) * GPC
        pk = pc[glo:ghi]
        chunk_id = np.empty(GPC, dtype=np.int64)
        local_start = np.empty(GPC, dtype=np.int64)
        cum = 0
        ch = 0
        for i in range(GPC):
            p = pk[i]
            if cum + p > C:
                ch += 1
                cum = 0
            chunk_id[i] = ch
            local_start[i] = cum
            cum += p
        packing.append((chunk_id, local_start))
        nchunk = max(nchunk, ch + 1)
    assert nchunk <= NCHUNK_CAP, f"needs {nchunk} chunks > {NCHUNK_CAP}"
    rt = nchunk * C
    npair = (nchunk + 1) // 2

    need_b3 = bool(np.any(tb != 0.0))
    nc = _build_bass(nchunk, need_b3)

    w1t = np.zeros((2 * MOL_C, HID_C), dtype=bf16)
    w1t[:MOL_C] = lw[:, :MOL_C].T.astype(bf16)
    w1t[MOL_C:] = w1t[:MOL_C]
    w2t = np.ascontiguousarray(lw[:, MOL_C:].T).astype(bf16)
    w3t = np.ascontiguousarray(tw.T).astype(bf16)
    b1v = np.ascontiguousarray(lb.reshape(HID_C, 1))
    b3v = np.ascontiguousarray(tb.reshape(HID_C, 1))

    in_maps = []
    ext = []
    for k in range(N_CORES):
        glo, ghi = k * GPC, (k + 1) * GPC
        ck = counts[glo:ghi]
        pk = pc[glo:ghi]
        chunk_id, local_start = packing[k]

        # destination rows for real node rows
        nk = int(ck.sum())
        dst_base = chunk_id * C + local_start
        src0 = row_begin[glo]
        within = np.arange(src0, src0 + nk) - np.repeat(row_begin[glo:ghi], ck)
        dst = np.repeat(dst_base, ck) + within
        # column permutation: row L of a chunk lands at column
        # (L//PAD) + (L%PAD)*DEC so the pair-reduce reads contiguous halves
        lc = dst % C
        dst = (dst - lc) + (lc // PAD) + (lc % PAD) * DEC

        # x1: chunk pairs stacked along the partition axis
        x1t = np.zeros((2 * MOL_C, npair * C), dtype=bf16)
        dch = dst // C
        dcol = (dch // 2) * C + (dst % C)
        drow = (dch % 2) * MOL_C
        x1v = x1[src0:src0 + nk].T.astype(bf16)       # [64, nk]
        even = drow == 0
        x1t[:MOL_C, dcol[even]] = x1v[:, even]
        x1t[MOL_C:, dcol[~even]] = x1v[:, ~even]

        x2t = np.zeros((HID_C, rt), dtype=bf16)
        x2t[:, dst] = x2[src0:src0 + nk].T.astype(bf16)

        in_maps.append({
            "x1t": x1t, "x2t": x2t, "w1t": w1t, "w2t": w2t, "w3t": w3t,
            "b1": b1v, "b3": b3v,
        })
        ext.append((ck, pk, chunk_id, local_start))

    res = run_bass_kernel_spmd(nc, in_maps, core_ids=list(range(N_CORES)))
    LAST_RESULTS = res

    # pad-row gated value (zero when biases are zero)
    pad_g = (1.0 / (1.0 + np.exp(-lb))) * tb          # [HID_C]

    out = np.empty((NUM_GRAPHS, HID_C), dtype=np.float32)
    for k in range(N_CORES):
        deck = np.asarray(res.results[k]["dec"])      # [HID_C, nchunk*DEC]
        ck, pk, chunk_id, local_start = ext[k]
        end_col = chunk_id * DEC + (local_start + pk) // PAD - 1
        start_col = chunk_id * DEC + local_start // PAD - 1
        e = deck[:, end_col]                          # [HID_C, GPC]
        s = deck[:, start_col]
        s[:, local_start == 0] = 0.0
        o = (e - s).T                                 # [GPC, HID_C]
        o -= (pk - ck)[:, None].astype(np.float32) * pad_g[None, :]
        o[ck == 0] = 0.0
        out[k * GPC:(k + 1) * GPC] = o
    return out
